# revision 42
# baseline (speedup 1.0000x reference)
"""DynamicProxyNCA loss on 8 TRN2 NeuronCores (Bass/Tile, SPMD) — class-blocked.

Key observation: the hardest-positive argmax for anchor i only ranges over
columns of i's own class (~132 of 8192). Anchors are class-sorted into 22
row-tiles of 128; each tile's candidate block = union of its (<=5) classes'
columns (<=655), padded to BLKW=1024. Each core owns 3 tiles; scores are just
3 x [128, 1024] instead of 30 x [128, 512] windows — ~5x less matmul and
max/argmax work than the windowed layout.

Score(i, j) = -2<p_i, z_j> (bf16 matmul) + [v_j - 512 + 512*classmatch]
(bf16 matmul: 8 local-class channels + v channel) + suffix mask
(block position < thr_i -> -1e4, where thr_i is the block position of
anchor i itself; within-class columns are sorted by original index, so the
suffix condition j >= i is a position threshold). Selection runs on
truncated-bf16 z; D_p / D_n are recomputed in f32 from exactly gathered
rows, keeping the final loss error ~3e-5.
"""
import sys

sys.path.insert(0, "/opt/trn_rl_repo")

import numpy as np
import ml_dtypes

import concourse.bass as bass
import concourse.tile as tile
from concourse import bacc, mybir
from concourse.bass_utils import run_bass_kernel_spmd
from concourse.masks import make_identity

F32 = mybir.dt.float32
F32R = mybir.dt.float32r
BF16 = mybir.dt.bfloat16
U32 = mybir.dt.uint32

B, Z = 8192, 128
NCLS = 62
P = 93
EPS = 1e-6
A = 2730                # anchors: range(0, 8189, 3)
RT = 128                # anchors per row tile
T = 22                  # row tiles
NCORE = 8
SLOTS = 3               # row tiles per core: c, c+8, c+16
BLKW = 768              # padded block width (2 col-tiles of 384)
WB = 2                  # col tiles per block
NLOC = 8                # max local classes per block
CT = BLKW // WB
SW = 672                # real scored width per block (max data block 655)
BIGNEG = -1.0e4
PEN = 512.0
EPS2 = 2.0 * EPS
ZEPS2 = Z * EPS * EPS
VROW = NLOC             # yext row of the v channel

_CACHE = {}


def build_program():
    nc = bacc.Bacc(None, target_bir_lowering=False, debug=False)

    zblk0 = nc.dram_tensor("zblk0", [Z, BLKW], BF16, kind="ExternalInput")
    zblk1 = nc.dram_tensor("zblk1", [Z, BLKW], BF16, kind="ExternalInput")
    zblk2 = nc.dram_tensor("zblk2", [Z, BLKW], BF16, kind="ExternalInput")
    yohbl = nc.dram_tensor("yohbl", [NLOC, SLOTS * BLKW], BF16, kind="ExternalInput")
    zrlb = nc.dram_tensor("zrlb", [SLOTS * BLKW, Z], F32, kind="ExternalInput")
    pybl = nc.dram_tensor("pybl", [NLOC + 1, SLOTS * RT], BF16, kind="ExternalInput")
    zatbl = nc.dram_tensor("zatbl", [Z, SLOTS * RT], BF16, kind="ExternalInput")
    thrl = nc.dram_tensor("thrl", [RT, SLOTS], F32, kind="ExternalInput")
    prx_in = nc.dram_tensor("prx", [P, Z], F32, kind="ExternalInput")
    iota93_in = nc.dram_tensor("iota93", [RT, P], F32, kind="ExternalInput")
    colsel_in = nc.dram_tensor("colsel", [Z, 256], F32R, kind="ExternalInput")
    out = nc.dram_tensor("out", [RT, 2 * SLOTS], F32, kind="ExternalOutput")

    AL = mybir.AluOpType
    AF = mybir.ActivationFunctionType
    AX = mybir.AxisListType

    from contextlib import ExitStack

    with tile.TileContext(nc) as tc, ExitStack() as ctx:
        singles = ctx.enter_context(tc.tile_pool(name="singles", bufs=1))

        # ---- queue-ordered input DMAs (sync: prep + blocks 0,2; gpsimd: 1)
        prx = singles.tile([P, Z], F32)
        nc.sync.dma_start(out=prx[:, :], in_=prx_in[:, :])
        zatb = singles.tile([Z, SLOTS * RT], BF16)
        nc.gpsimd.dma_start(out=zatb[:, :], in_=zatbl[:, :])
        zblk = singles.tile([Z, SLOTS * BLKW], BF16)
        nc.sync.dma_start(out=zblk[:, 0:BLKW], in_=zblk0[:, :])
        nc.gpsimd.dma_start(out=zblk[:, BLKW:2 * BLKW], in_=zblk1[:, :])
        nc.sync.dma_start(out=zblk[:, 2 * BLKW:3 * BLKW], in_=zblk2[:, :])
        colsel_r = singles.tile([Z, 256], F32R)
        nc.sync.dma_start(out=colsel_r[:, :], in_=colsel_in[:, :])
        iota93 = singles.tile([RT, P], F32)
        nc.sync.dma_start(out=iota93[:, :], in_=iota93_in[:, :])
        thr = singles.tile([RT, SLOTS], F32)
        nc.sync.dma_start(out=thr[:, :], in_=thrl[:, :])
        yext = singles.tile([NLOC + 1, SLOTS * BLKW], BF16)
        nc.gpsimd.dma_start(out=yext[0:NLOC, :], in_=yohbl[:, :])
        pext = singles.tile([NLOC + 1, SLOTS * RT], BF16)
        nc.sync.dma_start(out=pext[:, :], in_=pybl[:, :])

        iota1k = singles.tile([RT, SW], F32)
        nc.gpsimd.iota(iota1k[:, :], pattern=[[1, SW]], base=0,
                       channel_multiplier=0,
                       allow_small_or_imprecise_dtypes=True)

        identity = singles.tile([128, 128], F32)
        make_identity(nc, identity[:, :])
        onescol = singles.tile([1, 128], F32)
        nc.vector.memset(onescol[:, :], 1.0)

        # per-slot suffix mask: block position < thr_i -> BIGNEG
        maskadd = singles.tile([RT, SLOTS, SW], F32)
        for s in range(SLOTS):
            nc.vector.tensor_scalar(out=maskadd[:, s, :], in0=iota1k[:, :],
                                    scalar1=thr[:, s:s + 1], scalar2=BIGNEG,
                                    op0=AL.is_lt, op1=AL.mult)

        outbuf = singles.tile([RT, 2 * SLOTS], F32)

        # ---- proxy preprocessing
        mprxT = singles.tile([Z, P], F32)        # -2*prx_n.T (f32, epilogue)
        mprx_bf = singles.tile([Z, P], BF16)     # -2*prx_n.T (bf16, prelim)
        w_bcast = singles.tile([RT, P], F32)
        sb_bcast = singles.tile([RT, P], F32)
        prx_n_keep = singles.tile([P, Z], F32)
        with tc.tile_pool(name="setup_sb", bufs=1) as stp, \
             tc.tile_pool(name="setup_ps", bufs=1, space="PSUM") as stps:
            scratch = stp.tile([P, Z], F32)
            ss = stp.tile([P, 1], F32)
            nc.scalar.activation(out=scratch[:, :], in_=prx[:, :], func=AF.Square,
                                 accum_out=ss[:, :])
            norm = stp.tile([P, 1], F32)
            nc.scalar.activation(out=norm[:, :], in_=ss[:, :], func=AF.Sqrt)
            nc.vector.tensor_scalar_max(out=norm[:, :], in0=norm[:, :], scalar1=1e-12)
            rn = stp.tile([P, 1], F32)
            nc.vector.reciprocal(out=rn[:, :], in_=norm[:, :])
            prx_n = stp.tile([P, Z], F32)
            nc.vector.tensor_scalar_mul(out=prx_n[:, :], in0=prx[:, :], scalar1=rn[:, :])
            bb0 = stp.tile([P, 1], F32)
            nc.scalar.activation(out=scratch[:, :], in_=prx_n[:, :], func=AF.Square,
                                 accum_out=bb0[:, :])
            sb0 = stp.tile([P, 1], F32)
            nc.vector.tensor_reduce(out=sb0[:, :], in_=prx_n[:, :], axis=AX.X, op=AL.add)

            ps_t = stps.tile([Z, P], F32, tag="pst")
            nc.tensor.transpose(out=ps_t[:, :], in_=prx_n[:, :], identity=identity[:P, :P])
            nc.scalar.mul(out=mprxT[:, :], in_=ps_t[:, :], mul=-2.0)
            nc.vector.tensor_copy(out=mprx_bf[:, :], in_=mprxT[:, :])

            ps_r = stps.tile([1, P], F32, tag="psr")
            nc.tensor.transpose(out=ps_r[:, :], in_=bb0[:, :], identity=identity[:P, :P])
            bbrow = stp.tile([1, P], F32)
            nc.vector.tensor_copy(out=bbrow[:, :], in_=ps_r[:, :])
            ps_r2 = stps.tile([1, P], F32, tag="psr")
            nc.tensor.transpose(out=ps_r2[:, :], in_=sb0[:, :], identity=identity[:P, :P])
            sbrow = stp.tile([1, P], F32)
            nc.vector.tensor_copy(out=sbrow[:, :], in_=ps_r2[:, :])
            wrow = stp.tile([1, P], F32)
            nc.vector.scalar_tensor_tensor(
                out=wrow[:, :], in0=sbrow[:, :], scalar=-EPS2, in1=bbrow[:, :],
                op0=AL.mult, op1=AL.add)

            ps_b = stps.tile([RT, P], F32, tag="psb")
            nc.tensor.matmul(ps_b[:, :], lhsT=onescol[:, :], rhs=wrow[:, :],
                             start=True, stop=True)
            nc.vector.tensor_copy(out=w_bcast[:, :], in_=ps_b[:, :])
            ps_b2 = stps.tile([RT, P], F32, tag="psb")
            nc.tensor.matmul(ps_b2[:, :], lhsT=onescol[:, :], rhs=sbrow[:, :],
                             start=True, stop=True)
            nc.vector.tensor_copy(out=sb_bcast[:, :], in_=ps_b2[:, :])
            nc.vector.tensor_copy(out=prx_n_keep[:, :], in_=prx_n[:, :])

        slot_sb = ctx.enter_context(tc.tile_pool(name="slot_sb", bufs=2))
        ps_a = ctx.enter_context(tc.tile_pool(name="ps_a", bufs=1, space="PSUM"))

        # ---- prelim per slot: nearest proxy for each anchor (bf16 scores)
        prelim_out = [None] * SLOTS

        def prelim_slot(s):
            a0 = s * RT
            ps_e = ps_a.tile([RT, P], F32, tag="E")
            nc.tensor.matmul(ps_e[:, :], lhsT=zatb[:, a0:a0 + RT],
                             rhs=mprx_bf[:, :], start=True, stop=True)
            E = slot_sb.tile([RT, P], F32, tag="E")
            nc.vector.tensor_tensor(out=E[:, :], in0=ps_e[:, :], in1=w_bcast[:, :],
                                    op=AL.add)
            mmin = slot_sb.tile([RT, 1], F32, tag="mmin")
            nc.vector.tensor_reduce(out=mmin[:, :], in_=E[:, :], axis=AX.X, op=AL.min)
            eqm = slot_sb.tile([RT, P], F32, tag="eqm")
            nc.vector.tensor_scalar(out=eqm[:, :], in0=E[:, :], scalar1=mmin[:, :],
                                    scalar2=-(2.0 ** 20), op0=AL.is_equal, op1=AL.mult)
            scr = slot_sb.tile([RT, P], F32, tag="scr")
            kq = slot_sb.tile([RT, 1], F32, tag="kq")
            nc.vector.tensor_tensor(out=scr[:, :], in0=eqm[:, :], in1=iota93[:, :],
                                    op=AL.add)
            nc.vector.tensor_reduce(out=kq[:, :], in_=scr[:, :], axis=AX.X, op=AL.min)
            onehot = slot_sb.tile([RT, P], F32, tag=f"onehot{s}")
            nc.vector.tensor_scalar(out=onehot[:, :], in0=iota93[:, :], scalar1=kq[:, :],
                                    scalar2=2.0 ** 20, op0=AL.subtract, op1=AL.subtract)
            nc.vector.tensor_scalar(out=onehot[:, :], in0=onehot[:, :], scalar1=0.0,
                                    scalar2=None, op0=AL.is_equal)
            sp = slot_sb.tile([RT, 1], F32, tag=f"sp{s}")
            nc.vector.tensor_tensor(out=scr[:, :], in0=onehot[:, :],
                                    in1=sb_bcast[:, :], op=AL.mult)
            nc.vector.tensor_reduce(out=sp[:, :], in_=scr[:, :], axis=AX.X, op=AL.add)
            ps_t2 = ps_a.tile([P, RT], F32, tag="ohT")
            nc.tensor.transpose(out=ps_t2[:, :], in_=onehot[:, :], identity=identity[:, :])
            ohT = slot_sb.tile([P, RT], F32, tag="ohT")
            nc.vector.tensor_copy(out=ohT[:, :], in_=ps_t2[:, :])
            ps_pp = ps_a.tile([Z, RT], F32, tag="pp")
            nc.tensor.matmul(ps_pp[:, :], lhsT=prx_n_keep[:, :], rhs=ohT[:, :],
                             start=True, stop=True)
            mproxT_b = slot_sb.tile([Z, RT], BF16, tag=f"mproxT{s}")
            nc.scalar.mul(out=mproxT_b[:, :], in_=ps_pp[:, :], mul=-2.0)
            prelim_out[s] = (mproxT_b, onehot, sp)

        # ---- stats: v = zz - 512 per block column (6 col-tiles, one group)
        ssb = ctx.enter_context(tc.tile_pool(name="stats_sb", bufs=3))
        ssb1 = ctx.enter_context(tc.tile_pool(name="stats_sb1", bufs=1))
        sps = ctx.enter_context(tc.tile_pool(name="stats_ps", bufs=1, space="PSUM"))

        def stats_all():
            ps_zz = sps.tile([16, CT], F32, tag="zz")
            for ctm in range(6):
                sq = ssb.tile([Z, CT], F32R, tag="sq")
                nc.scalar.activation(out=sq[:, :], in_=zblk[:, ctm * CT:(ctm + 1) * CT],
                                     func=AF.Square)
                nc.tensor.matmul(ps_zz[:, :], lhsT=colsel_r[:, 16 * ctm:16 * ctm + 16],
                                 rhs=sq[:, :], start=(ctm == 0), stop=(ctm == 5))
            vbf = ssb1.tile([6, CT], BF16, tag="vbf")
            nc.vector.tensor_scalar_add(out=vbf[:, :], in0=ps_zz[0:6, :],
                                        scalar1=-PEN)
            with tc.tile_pool(name="vscr", bufs=1, space="DRAM") as vdp:
                vd = vdp.tile([1, 6 * CT], BF16)
                nc.sync.dma_start(
                    out=vd.rearrange("one (p f) -> (one p) f", p=6),
                    in_=vbf[:, :])
                nc.sync.dma_start(
                    out=yext[VROW:VROW + 1, 0:6 * CT],
                    in_=vd[:, :])

        prelim_slot(0)
        stats_all()
        prelim_slot(1)
        prelim_slot(2)

        ps_s = ctx.enter_context(tc.tile_pool(name="ps_s", bufs=2, space="PSUM"))

        # per-slot max/argmax records (single 672-wide pass per slot)
        tmax8 = [singles.tile([RT, 8], F32, name=f"tmax8_{s}")
                 for s in range(SLOTS)]
        tidx8 = [singles.tile([RT, 8], U32, name=f"tidx8_{s}")
                 for s in range(SLOTS)]
        ep = {}

        def strip_tile(s, f):
            del f
            a0 = s * RT
            mproxT_b = prelim_out[s][0]
            col = s * BLKW
            # 2-bank psum tile; halves written at bank-aligned offsets 0, 512
            ps = ps_s.tile([RT, 1024], F32, tag="S")
            nc.tensor.matmul(ps[:, 0:512], lhsT=mproxT_b[:, :],
                             rhs=zblk[:, col:col + 512], start=True, stop=False)
            nc.tensor.matmul(ps[:, 0:512], lhsT=pext[:, a0:a0 + RT],
                             rhs=yext[:, col:col + 512], start=False, stop=True)
            nc.tensor.matmul(ps[:, 512:512 + (SW - 512)], lhsT=mproxT_b[:, :],
                             rhs=zblk[:, col + 512:col + SW], start=True, stop=False)
            nc.tensor.matmul(ps[:, 512:512 + (SW - 512)], lhsT=pext[:, a0:a0 + RT],
                             rhs=yext[:, col + 512:col + SW], start=False, stop=True)
            nc.vector.tensor_tensor(out=ps[:, 0:SW], in0=ps[:, 0:SW],
                                    in1=maskadd[:, s, :], op=AL.add)
            nc.vector.max(tmax8[s][:, :], ps[:, 0:SW])
            nc.vector.max_index(out=tidx8[s][:, :], in_max=tmax8[s][:, :],
                                in_values=ps[:, 0:SW])

        def finalize_slot(s):
            mproxT_b, onehot, sp = prelim_out[s]
            m = slot_sb.tile([RT, 1], F32, tag=f"m{s}")
            nc.vector.tensor_copy(out=m[:, :], in_=tmax8[s][:, 0:1])
            jf = slot_sb.tile([RT, 1], F32, tag=f"jf{s}")
            nc.vector.tensor_copy(out=jf[:, :], in_=tidx8[s][:, 0:1])
            nc.vector.tensor_scalar_add(out=jf[:, :], in0=jf[:, :],
                                        scalar1=float(s * BLKW))
            ju = slot_sb.tile([RT, 1], U32, tag="ju")
            nc.vector.tensor_copy(out=ju[:, :], in_=jf[:, :])

            zp = slot_sb.tile([RT, Z], F32, tag="zp")
            nc.gpsimd.indirect_dma_start(
                out=zp[:, :], out_offset=None, in_=zrlb[:, :],
                in_offset=bass.IndirectOffsetOnAxis(ap=ju[:, 0:1], axis=0))
            zzjp = slot_sb.tile([RT, 1], F32, tag=f"zzjp{s}")
            scr2 = slot_sb.tile([RT, Z], F32, tag="scr2")
            nc.vector.tensor_tensor(out=scr2[:, :], in0=zp[:, :], in1=zp[:, :],
                                    op=AL.mult)
            nc.vector.tensor_reduce(out=zzjp[:, :], in_=scr2[:, :], axis=AX.X,
                                    op=AL.add)
            szjp = slot_sb.tile([RT, 1], F32, tag=f"szjp{s}")
            nc.vector.tensor_reduce(out=szjp[:, :], in_=zp[:, :], axis=AX.X, op=AL.add)
            ps_zt = ps_a.tile([Z, RT], F32, tag="pp")
            nc.tensor.transpose(out=ps_zt[:, :], in_=zp[:, :], identity=identity[:, :])
            zpT = slot_sb.tile([Z, RT], F32, tag="zpT")
            nc.vector.tensor_copy(out=zpT[:, :], in_=ps_zt[:, :])
            ps_dn = ps_a.tile([RT, P], F32, tag="E")
            nc.tensor.matmul(ps_dn[:, :], lhsT=zpT[:, :], rhs=mprxT[:, :],
                             start=True, stop=True)
            zc = slot_sb.tile([RT, 1], F32, tag="zc")
            nc.vector.tensor_scalar(out=zc[:, :], in0=szjp[:, :], scalar1=EPS2,
                                    scalar2=ZEPS2, op0=AL.mult, op1=AL.add)
            nc.vector.tensor_tensor(out=zc[:, :], in0=zc[:, :], in1=zzjp[:, :],
                                    op=AL.add)
            dn2 = slot_sb.tile([RT, P], F32, tag=f"dn2{s}")
            nc.vector.scalar_tensor_tensor(
                out=dn2[:, :], in0=ps_dn[:, :], scalar=zc[:, :], in1=w_bcast[:, :],
                op0=AL.add, op1=AL.add)
            scr = slot_sb.tile([RT, P], F32, tag="scr")
            dsel = slot_sb.tile([RT, 1], F32, tag=f"dsel{s}")
            nc.vector.tensor_tensor(out=scr[:, :], in0=dn2[:, :], in1=onehot[:, :],
                                    op=AL.mult)
            nc.vector.tensor_reduce(out=dsel[:, :], in_=scr[:, :], axis=AX.X,
                                    op=AL.add)
            nc.vector.tensor_scalar_max(out=dn2[:, :], in0=dn2[:, :], scalar1=0.0)
            nc.vector.tensor_copy(out=outbuf[:, 2 * s:2 * s + 1], in_=m[:, :])
            ep[s] = (dn2, dsel, zzjp, szjp)

        for s in range(SLOTS):
            strip_tile(s, 0)
            finalize_slot(s)

        # ---- epilogue: dp math into a shared batch, one Sqrt pass over
        # [dn2 x3 | dp x3], per-slot Exp (accumulated), one Ln batch
        batch = singles.tile([RT, 3 * P + 3], F32)
        for s in range(SLOTS):
            _, dsel, zzjp, szjp = ep[s]
            sp = prelim_out[s][2]
            dp = slot_sb.tile([RT, 1], F32, tag=f"dp{s}")
            nc.vector.tensor_tensor(out=dp[:, :], in0=sp[:, :], in1=szjp[:, :],
                                    op=AL.subtract)
            nc.vector.scalar_tensor_tensor(
                out=dp[:, :], in0=dp[:, :], scalar=4.0 * EPS, in1=dsel[:, :],
                op0=AL.mult, op1=AL.add)
            nc.vector.tensor_scalar_max(
                out=batch[:, 3 * P + s:3 * P + s + 1], in0=dp[:, :], scalar1=0.0)
            nc.vector.tensor_copy(out=batch[:, s * P:(s + 1) * P],
                                  in_=ep[s][0][:, :])
        broot = singles.tile([RT, 3 * P + 3], F32)
        nc.scalar.activation(out=broot[:, :], in_=batch[:, :], func=AF.Sqrt)
        sumes = singles.tile([RT, 3], F32)
        for s in range(SLOTS):
            expd = slot_sb.tile([RT, P], F32, tag="expd")
            nc.scalar.activation(out=expd[:, :], in_=broot[:, s * P:(s + 1) * P],
                                 func=AF.Exp, scale=-1.0,
                                 accum_out=sumes[:, s:s + 1])
        lses = singles.tile([RT, 3], F32)
        nc.scalar.activation(out=lses[:, :], in_=sumes[:, :], func=AF.Ln)
        for s in range(SLOTS):
            nc.vector.tensor_tensor(out=outbuf[:, 2 * s + 1:2 * s + 2],
                                    in0=broot[:, 3 * P + s:3 * P + s + 1],
                                    in1=lses[:, s:s + 1], op=AL.add)

        nc.sync.dma_start(out=out[:, :], in_=outbuf[:, :])

    nc.finalize()
    return nc


def prep_inputs(z, y_idx, proxies, y_map):
    """Host-side sharding/layout prep: class-sorted anchors, per-tile class
    column blocks, thresholds. Only float transformation is byte-truncation
    f32 -> bf16 (no arithmetic)."""
    bf16 = ml_dtypes.bfloat16
    z = np.ascontiguousarray(np.asarray(z, dtype=np.float32))
    y = np.asarray(y_idx, dtype=np.int32)
    y_map = np.asarray(y_map, dtype=np.int32)
    lut = np.zeros(int(y_map.max()) + 1, dtype=np.int32)
    lut[y_map] = np.arange(len(y_map), dtype=np.int32)
    yrel = lut[y]

    zT = np.ascontiguousarray(z.T)                       # [Z, B] f32
    zTbf = zT.view(np.uint16)[:, 1::2].copy().view(bf16)  # truncated bf16

    anchors = np.arange(0, B - 3, 3, dtype=np.int64)
    ya = yrel[anchors]
    order = np.argsort(ya, kind="stable")
    aso = anchors[order]                                  # class-sorted anchors
    yso = ya[order]
    cols_by_class = [np.flatnonzero(yrel == cl) for cl in range(NCLS)]

    tiles = []
    for t in range(T):
        rows = aso[RT * t:RT * t + RT]
        rcls = yso[RT * t:RT * t + RT]
        cls_order = list(dict.fromkeys(int(c) for c in rcls))
        assert len(cls_order) <= NLOC
        blockcols = np.concatenate([cols_by_class[cl] for cl in cls_order])
        assert len(blockcols) <= BLKW, len(blockcols)
        posmap = {int(cj): p for p, cj in enumerate(blockcols)}
        thr_t = np.array([posmap[int(a)] for a in rows], dtype=np.float32)
        lcls = {cl: lc for lc, cl in enumerate(cls_order)}
        tiles.append((rows, rcls, cls_order, blockcols, thr_t, lcls))

    iota93 = np.broadcast_to(np.arange(P, dtype=np.float32), (RT, P)).copy()
    colsel = np.zeros((Z, 256), dtype=np.float32)
    for ct in range(16):
        colsel[:, 16 * ct + ct] = 1.0

    in_maps = []
    for c in range(NCORE):
        zblks = [np.zeros((Z, BLKW), dtype=bf16) for _ in range(SLOTS)]
        yohb = np.zeros((NLOC, SLOTS * BLKW), dtype=bf16)
        zrlb = np.zeros((SLOTS * BLKW, Z), dtype=np.float32)
        pyb = np.zeros((NLOC + 1, SLOTS * RT), dtype=bf16)
        pyb[NLOC, :] = bf16(1.0)
        zatb = np.zeros((Z, SLOTS * RT), dtype=bf16)
        thr = np.zeros((RT, SLOTS), dtype=np.float32)
        for s in range(SLOTS):
            t = c + 8 * s
            if t >= T:
                continue
            rows, rcls, cls_order, blockcols, thr_t, lcls = tiles[t]
            nb = len(blockcols)
            nk = len(rows)
            zblks[s][:, :nb] = zTbf[:, blockcols]
            for p_, cj in enumerate(blockcols):
                yohb[lcls[int(yrel[cj])], s * BLKW + p_] = bf16(1.0)
            zrlb[s * BLKW:s * BLKW + nb, :] = z[blockcols, :]
            pyb[[lcls[int(cl)] for cl in rcls], s * RT + np.arange(nk)] = bf16(PEN)
            zatb[:, s * RT:s * RT + nk] = zTbf[:, rows]
            thr[:nk, s] = thr_t
        in_maps.append({
            "zblk0": zblks[0], "zblk1": zblks[1], "zblk2": zblks[2],
            "yohbl": yohb, "zrlb": zrlb, "pybl": pyb, "zatbl": zatb,
            "thrl": thr, "prx": np.asarray(proxies, dtype=np.float32),
            "iota93": iota93, "colsel": colsel,
        })
    return in_maps


def combine(results):
    total = 0.0
    for t in range(T):
        c, s = t % 8, t // 8
        nk = min(RT, A - RT * t)
        loss = results[c]["out"][:nk, 2 * s + 1].astype(np.float64)
        total += loss.sum()
    return np.float32(total / A)


def kernel(z, y_idx, proxies, y_map, _trace=False):
    if "nc" not in _CACHE:
        _CACHE["nc"] = build_program()
    nc = _CACHE["nc"]
    in_maps = prep_inputs(z, y_idx, proxies, y_map)
    res = run_bass_kernel_spmd(nc, in_maps, core_ids=list(range(NCORE)),
                               trace=_trace)
    out = combine(res.results)
    if _trace:
        return out, res
    return out


if __name__ == "__main__":
    import jax
    with jax.default_device(jax.devices("cpu")[0]):
        import reference
        inputs = {k: np.asarray(v) for k, v in reference.setup_inputs().items()}
        expected = np.asarray(jax.jit(reference.reference, backend="cpu")(**inputs))
    actual = kernel(**inputs)
    rel = abs(float(actual) - float(expected)) / max(abs(float(expected)), 1e-12)
    print(f"expected {expected}, actual {actual}, rel err {rel:.3e}")


# revision 43
# speedup vs baseline: 1.2494x; 1.2494x over previous
"""DynamicProxyNCA loss on 8 TRN2 NeuronCores (Bass/Tile, SPMD) — class-blocked.

Key observation: the hardest-positive argmax for anchor i only ranges over
columns of i's own class (~132 of 8192). Anchors are class-sorted into 22
row-tiles of 128; each tile's candidate block = union of its (<=5) classes'
columns (<=655), padded to BLKW=1024. Each core owns 3 tiles; scores are just
3 x [128, 1024] instead of 30 x [128, 512] windows — ~5x less matmul and
max/argmax work than the windowed layout.

Score(i, j) = -2<p_i, z_j> (bf16 matmul) + [v_j - 512 + 512*classmatch]
(bf16 matmul: 8 local-class channels + v channel) + suffix mask
(block position < thr_i -> -1e4, where thr_i is the block position of
anchor i itself; within-class columns are sorted by original index, so the
suffix condition j >= i is a position threshold). Selection runs on
truncated-bf16 z; D_p / D_n are recomputed in f32 from exactly gathered
rows, keeping the final loss error ~3e-5.
"""
import sys

sys.path.insert(0, "/opt/trn_rl_repo")

import numpy as np
import ml_dtypes

import concourse.bass as bass
import concourse.tile as tile
from concourse import bacc, mybir
from concourse.bass_utils import run_bass_kernel_spmd
from concourse.masks import make_identity

F32 = mybir.dt.float32
F32R = mybir.dt.float32r
BF16 = mybir.dt.bfloat16
U32 = mybir.dt.uint32

B, Z = 8192, 128
NCLS = 62
P = 93
EPS = 1e-6
A = 2730                # anchors: range(0, 8189, 3)
RT = 128                # anchors per row tile
T = 22                  # row tiles
NCORE = 8
SLOTS = 3               # row tiles per core: c, c+8, c+16
BLKW = 768              # padded block width (2 col-tiles of 384)
WB = 2                  # col tiles per block
NLOC = 8                # max local classes per block
CT = BLKW // WB
BIGNEG = -1.0e4
PEN = 512.0
EPS2 = 2.0 * EPS
ZEPS2 = Z * EPS * EPS
VROW = NLOC             # yext row of the v channel

_CACHE = {}


def build_program():
    nc = bacc.Bacc(None, target_bir_lowering=False, debug=False)

    zblk0 = nc.dram_tensor("zblk0", [Z, BLKW], BF16, kind="ExternalInput")
    zblk1 = nc.dram_tensor("zblk1", [Z, BLKW], BF16, kind="ExternalInput")
    zblk2 = nc.dram_tensor("zblk2", [Z, BLKW], BF16, kind="ExternalInput")
    yohbl = nc.dram_tensor("yohbl", [NLOC, SLOTS * BLKW], BF16, kind="ExternalInput")
    zrlb = nc.dram_tensor("zrlb", [SLOTS * BLKW, Z], F32, kind="ExternalInput")
    pybl = nc.dram_tensor("pybl", [NLOC + 1, SLOTS * RT], BF16, kind="ExternalInput")
    zatbl = nc.dram_tensor("zatbl", [Z, SLOTS * RT], BF16, kind="ExternalInput")
    thrl = nc.dram_tensor("thrl", [RT, SLOTS], F32, kind="ExternalInput")
    prx_in = nc.dram_tensor("prx", [P, Z], F32, kind="ExternalInput")
    iota93_in = nc.dram_tensor("iota93", [RT, P], F32, kind="ExternalInput")
    colsel_in = nc.dram_tensor("colsel", [Z, 256], F32R, kind="ExternalInput")
    out = nc.dram_tensor("out", [RT, 2 * SLOTS], F32, kind="ExternalOutput")

    AL = mybir.AluOpType
    AF = mybir.ActivationFunctionType
    AX = mybir.AxisListType

    from contextlib import ExitStack

    with tile.TileContext(nc) as tc, ExitStack() as ctx:
        singles = ctx.enter_context(tc.tile_pool(name="singles", bufs=1))

        # ---- queue-ordered input DMAs (sync: prep + blocks 0,2; gpsimd: 1)
        prx = singles.tile([P, Z], F32)
        nc.sync.dma_start(out=prx[:, :], in_=prx_in[:, :])
        zatb = singles.tile([Z, SLOTS * RT], BF16)
        nc.gpsimd.dma_start(out=zatb[:, :], in_=zatbl[:, :])
        zblk = singles.tile([Z, SLOTS * BLKW], BF16)
        nc.sync.dma_start(out=zblk[:, 0:BLKW], in_=zblk0[:, :])
        nc.gpsimd.dma_start(out=zblk[:, BLKW:2 * BLKW], in_=zblk1[:, :])
        nc.sync.dma_start(out=zblk[:, 2 * BLKW:3 * BLKW], in_=zblk2[:, :])
        colsel_r = singles.tile([Z, 256], F32R)
        nc.sync.dma_start(out=colsel_r[:, :], in_=colsel_in[:, :])
        iota93 = singles.tile([RT, P], F32)
        nc.sync.dma_start(out=iota93[:, :], in_=iota93_in[:, :])
        thr = singles.tile([RT, SLOTS], F32)
        nc.sync.dma_start(out=thr[:, :], in_=thrl[:, :])
        yext = singles.tile([NLOC + 1, SLOTS * BLKW], BF16)
        nc.gpsimd.dma_start(out=yext[0:NLOC, :], in_=yohbl[:, :])
        pext = singles.tile([NLOC + 1, SLOTS * RT], BF16)
        nc.sync.dma_start(out=pext[:, :], in_=pybl[:, :])

        iota1k = singles.tile([RT, BLKW], F32)
        nc.gpsimd.iota(iota1k[:, :], pattern=[[1, BLKW]], base=0,
                       channel_multiplier=0,
                       allow_small_or_imprecise_dtypes=True)

        identity = singles.tile([128, 128], F32)
        make_identity(nc, identity[:, :])
        onescol = singles.tile([1, 128], F32)
        nc.vector.memset(onescol[:, :], 1.0)

        # per-slot suffix mask: block position < thr_i -> BIGNEG
        maskadd = singles.tile([RT, SLOTS, BLKW], F32)
        for s in range(SLOTS):
            nc.vector.tensor_scalar(out=maskadd[:, s, :], in0=iota1k[:, :],
                                    scalar1=thr[:, s:s + 1], scalar2=BIGNEG,
                                    op0=AL.is_lt, op1=AL.mult)

        outbuf = singles.tile([RT, 2 * SLOTS], F32)

        # ---- proxy preprocessing
        mprxT = singles.tile([Z, P], F32)        # -2*prx_n.T (f32, epilogue)
        mprx_bf = singles.tile([Z, P], BF16)     # -2*prx_n.T (bf16, prelim)
        w_bcast = singles.tile([RT, P], F32)
        sb_bcast = singles.tile([RT, P], F32)
        prx_n_keep = singles.tile([P, Z], F32)
        with tc.tile_pool(name="setup_sb", bufs=1) as stp, \
             tc.tile_pool(name="setup_ps", bufs=1, space="PSUM") as stps:
            scratch = stp.tile([P, Z], F32)
            ss = stp.tile([P, 1], F32)
            nc.scalar.activation(out=scratch[:, :], in_=prx[:, :], func=AF.Square,
                                 accum_out=ss[:, :])
            norm = stp.tile([P, 1], F32)
            nc.scalar.activation(out=norm[:, :], in_=ss[:, :], func=AF.Sqrt)
            nc.vector.tensor_scalar_max(out=norm[:, :], in0=norm[:, :], scalar1=1e-12)
            rn = stp.tile([P, 1], F32)
            nc.vector.reciprocal(out=rn[:, :], in_=norm[:, :])
            prx_n = stp.tile([P, Z], F32)
            nc.vector.tensor_scalar_mul(out=prx_n[:, :], in0=prx[:, :], scalar1=rn[:, :])
            bb0 = stp.tile([P, 1], F32)
            nc.scalar.activation(out=scratch[:, :], in_=prx_n[:, :], func=AF.Square,
                                 accum_out=bb0[:, :])
            sb0 = stp.tile([P, 1], F32)
            nc.vector.tensor_reduce(out=sb0[:, :], in_=prx_n[:, :], axis=AX.X, op=AL.add)

            ps_t = stps.tile([Z, P], F32, tag="pst")
            nc.tensor.transpose(out=ps_t[:, :], in_=prx_n[:, :], identity=identity[:P, :P])
            nc.scalar.mul(out=mprxT[:, :], in_=ps_t[:, :], mul=-2.0)
            nc.vector.tensor_copy(out=mprx_bf[:, :], in_=mprxT[:, :])

            ps_r = stps.tile([1, P], F32, tag="psr")
            nc.tensor.transpose(out=ps_r[:, :], in_=bb0[:, :], identity=identity[:P, :P])
            bbrow = stp.tile([1, P], F32)
            nc.vector.tensor_copy(out=bbrow[:, :], in_=ps_r[:, :])
            ps_r2 = stps.tile([1, P], F32, tag="psr")
            nc.tensor.transpose(out=ps_r2[:, :], in_=sb0[:, :], identity=identity[:P, :P])
            sbrow = stp.tile([1, P], F32)
            nc.vector.tensor_copy(out=sbrow[:, :], in_=ps_r2[:, :])
            wrow = stp.tile([1, P], F32)
            nc.vector.scalar_tensor_tensor(
                out=wrow[:, :], in0=sbrow[:, :], scalar=-EPS2, in1=bbrow[:, :],
                op0=AL.mult, op1=AL.add)

            ps_b = stps.tile([RT, P], F32, tag="psb")
            nc.tensor.matmul(ps_b[:, :], lhsT=onescol[:, :], rhs=wrow[:, :],
                             start=True, stop=True)
            nc.vector.tensor_copy(out=w_bcast[:, :], in_=ps_b[:, :])
            ps_b2 = stps.tile([RT, P], F32, tag="psb")
            nc.tensor.matmul(ps_b2[:, :], lhsT=onescol[:, :], rhs=sbrow[:, :],
                             start=True, stop=True)
            nc.vector.tensor_copy(out=sb_bcast[:, :], in_=ps_b2[:, :])
            nc.vector.tensor_copy(out=prx_n_keep[:, :], in_=prx_n[:, :])

        slot_sb = ctx.enter_context(tc.tile_pool(name="slot_sb", bufs=2))
        ps_a = ctx.enter_context(tc.tile_pool(name="ps_a", bufs=1, space="PSUM"))

        # ---- prelim per slot: nearest proxy for each anchor (bf16 scores)
        prelim_out = [None] * SLOTS

        def prelim_slot(s):
            a0 = s * RT
            ps_e = ps_a.tile([RT, P], F32, tag="E")
            nc.tensor.matmul(ps_e[:, :], lhsT=zatb[:, a0:a0 + RT],
                             rhs=mprx_bf[:, :], start=True, stop=True)
            E = slot_sb.tile([RT, P], F32, tag="E")
            nc.vector.tensor_tensor(out=E[:, :], in0=ps_e[:, :], in1=w_bcast[:, :],
                                    op=AL.add)
            mmin = slot_sb.tile([RT, 1], F32, tag="mmin")
            nc.vector.tensor_reduce(out=mmin[:, :], in_=E[:, :], axis=AX.X, op=AL.min)
            eqm = slot_sb.tile([RT, P], F32, tag="eqm")
            nc.vector.tensor_scalar(out=eqm[:, :], in0=E[:, :], scalar1=mmin[:, :],
                                    scalar2=-(2.0 ** 20), op0=AL.is_equal, op1=AL.mult)
            scr = slot_sb.tile([RT, P], F32, tag="scr")
            kq = slot_sb.tile([RT, 1], F32, tag="kq")
            nc.vector.tensor_tensor(out=scr[:, :], in0=eqm[:, :], in1=iota93[:, :],
                                    op=AL.add)
            nc.vector.tensor_reduce(out=kq[:, :], in_=scr[:, :], axis=AX.X, op=AL.min)
            onehot = slot_sb.tile([RT, P], F32, tag=f"onehot{s}")
            nc.vector.tensor_scalar(out=onehot[:, :], in0=iota93[:, :], scalar1=kq[:, :],
                                    scalar2=2.0 ** 20, op0=AL.subtract, op1=AL.subtract)
            nc.vector.tensor_scalar(out=onehot[:, :], in0=onehot[:, :], scalar1=0.0,
                                    scalar2=None, op0=AL.is_equal)
            sp = slot_sb.tile([RT, 1], F32, tag=f"sp{s}")
            nc.vector.tensor_tensor(out=scr[:, :], in0=onehot[:, :],
                                    in1=sb_bcast[:, :], op=AL.mult)
            nc.vector.tensor_reduce(out=sp[:, :], in_=scr[:, :], axis=AX.X, op=AL.add)
            ps_t2 = ps_a.tile([P, RT], F32, tag="ohT")
            nc.tensor.transpose(out=ps_t2[:, :], in_=onehot[:, :], identity=identity[:, :])
            ohT = slot_sb.tile([P, RT], F32, tag="ohT")
            nc.vector.tensor_copy(out=ohT[:, :], in_=ps_t2[:, :])
            ps_pp = ps_a.tile([Z, RT], F32, tag="pp")
            nc.tensor.matmul(ps_pp[:, :], lhsT=prx_n_keep[:, :], rhs=ohT[:, :],
                             start=True, stop=True)
            mproxT_b = slot_sb.tile([Z, RT], BF16, tag=f"mproxT{s}")
            nc.scalar.mul(out=mproxT_b[:, :], in_=ps_pp[:, :], mul=-2.0)
            prelim_out[s] = (mproxT_b, onehot, sp)

        # ---- stats: v = zz - 512 per block column (6 col-tiles, one group)
        ssb = ctx.enter_context(tc.tile_pool(name="stats_sb", bufs=3))
        ssb1 = ctx.enter_context(tc.tile_pool(name="stats_sb1", bufs=1))
        sps = ctx.enter_context(tc.tile_pool(name="stats_ps", bufs=1, space="PSUM"))

        def stats_all():
            ps_zz = sps.tile([16, CT], F32, tag="zz")
            for ctm in range(6):
                sq = ssb.tile([Z, CT], F32R, tag="sq")
                nc.scalar.activation(out=sq[:, :], in_=zblk[:, ctm * CT:(ctm + 1) * CT],
                                     func=AF.Square)
                nc.tensor.matmul(ps_zz[:, :], lhsT=colsel_r[:, 16 * ctm:16 * ctm + 16],
                                 rhs=sq[:, :], start=(ctm == 0), stop=(ctm == 5))
            vbf = ssb1.tile([6, CT], BF16, tag="vbf")
            nc.vector.tensor_scalar_add(out=vbf[:, :], in0=ps_zz[0:6, :],
                                        scalar1=-PEN)
            with tc.tile_pool(name="vscr", bufs=1, space="DRAM") as vdp:
                vd = vdp.tile([1, 6 * CT], BF16)
                nc.sync.dma_start(
                    out=vd.rearrange("one (p f) -> (one p) f", p=6),
                    in_=vbf[:, :])
                nc.sync.dma_start(
                    out=yext[VROW:VROW + 1, 0:6 * CT],
                    in_=vd[:, :])

        prelim_slot(0)
        stats_all()
        prelim_slot(1)
        prelim_slot(2)

        ps_s = ctx.enter_context(tc.tile_pool(name="ps_s", bufs=4, space="PSUM"))

        # per-slot per-tile max/argmax records
        tmax8 = [singles.tile([RT, 8 * WB], F32, name=f"tmax8_{s}")
                 for s in range(SLOTS)]
        tidx8 = [singles.tile([RT, 8 * WB], U32, name=f"tidx8_{s}")
                 for s in range(SLOTS)]
        ep = {}

        def strip_tile(s, f):
            a0 = s * RT
            mproxT_b = prelim_out[s][0]
            col = s * BLKW + f * CT
            ps = ps_s.tile([RT, CT], F32, tag="S")
            nc.tensor.matmul(ps[:, :], lhsT=mproxT_b[:, :],
                             rhs=zblk[:, col:col + CT], start=True, stop=False)
            nc.tensor.matmul(ps[:, :], lhsT=pext[:, a0:a0 + RT],
                             rhs=yext[:, col:col + CT], start=False, stop=True)
            nc.vector.tensor_tensor(out=ps[:, :], in0=ps[:, :],
                                    in1=maskadd[:, s, f * CT:(f + 1) * CT],
                                    op=AL.add)
            nc.vector.max(tmax8[s][:, 8 * f:8 * f + 8], ps[:, :])
            nc.vector.max_index(out=tidx8[s][:, 8 * f:8 * f + 8],
                                in_max=tmax8[s][:, 8 * f:8 * f + 8],
                                in_values=ps[:, :])

        def finalize_slot(s):
            mproxT_b, onehot, sp = prelim_out[s]
            tv = tmax8[s][:, 0:8 * WB:8]
            m = slot_sb.tile([RT, 1], F32, tag=f"m{s}")
            nc.vector.tensor_tensor(out=m[:, :], in0=tv[:, 0:1], in1=tv[:, 1:2],
                                    op=AL.max)
            # strict >: ties pick tile 0 (first occurrence, matches argmax)
            sel = slot_sb.tile([RT, 1], F32, tag="sel")
            nc.vector.tensor_tensor(out=sel[:, :], in0=tv[:, 1:2], in1=tv[:, 0:1],
                                    op=AL.is_gt)
            tidxf = slot_sb.tile([RT, WB], F32, tag="tidxf")
            nc.vector.tensor_copy(out=tidxf[:, :], in_=tidx8[s][:, 0:8 * WB:8])
            diff = slot_sb.tile([RT, 1], F32, tag="diff")
            nc.vector.tensor_tensor(out=diff[:, :], in0=tidxf[:, 1:2],
                                    in1=tidxf[:, 0:1], op=AL.subtract)
            nc.vector.tensor_scalar_add(out=diff[:, :], in0=diff[:, :],
                                        scalar1=float(CT))
            jf = slot_sb.tile([RT, 1], F32, tag=f"jf{s}")
            nc.vector.tensor_tensor(out=jf[:, :], in0=sel[:, :], in1=diff[:, :],
                                    op=AL.mult)
            nc.vector.tensor_tensor(out=jf[:, :], in0=jf[:, :], in1=tidxf[:, 0:1],
                                    op=AL.add)
            nc.vector.tensor_scalar_add(out=jf[:, :], in0=jf[:, :],
                                        scalar1=float(s * BLKW))
            ju = slot_sb.tile([RT, 1], U32, tag="ju")
            nc.vector.tensor_copy(out=ju[:, :], in_=jf[:, :])

            zp = slot_sb.tile([RT, Z], F32, tag="zp")
            nc.gpsimd.indirect_dma_start(
                out=zp[:, :], out_offset=None, in_=zrlb[:, :],
                in_offset=bass.IndirectOffsetOnAxis(ap=ju[:, 0:1], axis=0))
            zzjp = slot_sb.tile([RT, 1], F32, tag=f"zzjp{s}")
            scr2 = slot_sb.tile([RT, Z], F32, tag="scr2")
            nc.vector.tensor_tensor(out=scr2[:, :], in0=zp[:, :], in1=zp[:, :],
                                    op=AL.mult)
            nc.vector.tensor_reduce(out=zzjp[:, :], in_=scr2[:, :], axis=AX.X,
                                    op=AL.add)
            szjp = slot_sb.tile([RT, 1], F32, tag=f"szjp{s}")
            nc.vector.tensor_reduce(out=szjp[:, :], in_=zp[:, :], axis=AX.X, op=AL.add)
            ps_zt = ps_a.tile([Z, RT], F32, tag="pp")
            nc.tensor.transpose(out=ps_zt[:, :], in_=zp[:, :], identity=identity[:, :])
            zpT = slot_sb.tile([Z, RT], F32, tag="zpT")
            nc.vector.tensor_copy(out=zpT[:, :], in_=ps_zt[:, :])
            ps_dn = ps_a.tile([RT, P], F32, tag="E")
            nc.tensor.matmul(ps_dn[:, :], lhsT=zpT[:, :], rhs=mprxT[:, :],
                             start=True, stop=True)
            zc = slot_sb.tile([RT, 1], F32, tag="zc")
            nc.vector.tensor_scalar(out=zc[:, :], in0=szjp[:, :], scalar1=EPS2,
                                    scalar2=ZEPS2, op0=AL.mult, op1=AL.add)
            nc.vector.tensor_tensor(out=zc[:, :], in0=zc[:, :], in1=zzjp[:, :],
                                    op=AL.add)
            dn2 = slot_sb.tile([RT, P], F32, tag=f"dn2{s}")
            nc.vector.scalar_tensor_tensor(
                out=dn2[:, :], in0=ps_dn[:, :], scalar=zc[:, :], in1=w_bcast[:, :],
                op0=AL.add, op1=AL.add)
            scr = slot_sb.tile([RT, P], F32, tag="scr")
            dsel = slot_sb.tile([RT, 1], F32, tag=f"dsel{s}")
            nc.vector.tensor_tensor(out=scr[:, :], in0=dn2[:, :], in1=onehot[:, :],
                                    op=AL.mult)
            nc.vector.tensor_reduce(out=dsel[:, :], in_=scr[:, :], axis=AX.X,
                                    op=AL.add)
            nc.vector.tensor_scalar_max(out=dn2[:, :], in0=dn2[:, :], scalar1=0.0)
            nc.vector.tensor_copy(out=outbuf[:, 2 * s:2 * s + 1], in_=m[:, :])
            ep[s] = (dn2, dsel, zzjp, szjp)

        for s in range(SLOTS):
            for f in range(WB):
                strip_tile(s, f)
            finalize_slot(s)

        # ---- epilogue: dp math into a shared batch, one Sqrt pass over
        # [dn2 x3 | dp x3], per-slot Exp (accumulated), one Ln batch
        batch = singles.tile([RT, 3 * P + 3], F32)
        for s in range(SLOTS):
            _, dsel, zzjp, szjp = ep[s]
            sp = prelim_out[s][2]
            dp = slot_sb.tile([RT, 1], F32, tag=f"dp{s}")
            nc.vector.tensor_tensor(out=dp[:, :], in0=sp[:, :], in1=szjp[:, :],
                                    op=AL.subtract)
            nc.vector.scalar_tensor_tensor(
                out=dp[:, :], in0=dp[:, :], scalar=4.0 * EPS, in1=dsel[:, :],
                op0=AL.mult, op1=AL.add)
            nc.vector.tensor_scalar_max(
                out=batch[:, 3 * P + s:3 * P + s + 1], in0=dp[:, :], scalar1=0.0)
            nc.vector.tensor_copy(out=batch[:, s * P:(s + 1) * P],
                                  in_=ep[s][0][:, :])
        broot = singles.tile([RT, 3 * P + 3], F32)
        nc.scalar.activation(out=broot[:, :], in_=batch[:, :], func=AF.Sqrt)
        sumes = singles.tile([RT, 3], F32)
        for s in range(SLOTS):
            expd = slot_sb.tile([RT, P], F32, tag="expd")
            nc.scalar.activation(out=expd[:, :], in_=broot[:, s * P:(s + 1) * P],
                                 func=AF.Exp, scale=-1.0,
                                 accum_out=sumes[:, s:s + 1])
        lses = singles.tile([RT, 3], F32)
        nc.scalar.activation(out=lses[:, :], in_=sumes[:, :], func=AF.Ln)
        for s in range(SLOTS):
            nc.vector.tensor_tensor(out=outbuf[:, 2 * s + 1:2 * s + 2],
                                    in0=broot[:, 3 * P + s:3 * P + s + 1],
                                    in1=lses[:, s:s + 1], op=AL.add)

        nc.sync.dma_start(out=out[:, :], in_=outbuf[:, :])

    nc.finalize()
    return nc


def prep_inputs(z, y_idx, proxies, y_map):
    """Host-side sharding/layout prep: class-sorted anchors, per-tile class
    column blocks, thresholds. Only float transformation is byte-truncation
    f32 -> bf16 (no arithmetic)."""
    bf16 = ml_dtypes.bfloat16
    z = np.ascontiguousarray(np.asarray(z, dtype=np.float32))
    y = np.asarray(y_idx, dtype=np.int32)
    y_map = np.asarray(y_map, dtype=np.int32)
    lut = np.zeros(int(y_map.max()) + 1, dtype=np.int32)
    lut[y_map] = np.arange(len(y_map), dtype=np.int32)
    yrel = lut[y]

    zT = np.ascontiguousarray(z.T)                       # [Z, B] f32
    zTbf = zT.view(np.uint16)[:, 1::2].copy().view(bf16)  # truncated bf16

    anchors = np.arange(0, B - 3, 3, dtype=np.int64)
    ya = yrel[anchors]
    order = np.argsort(ya, kind="stable")
    aso = anchors[order]                                  # class-sorted anchors
    yso = ya[order]
    cols_by_class = [np.flatnonzero(yrel == cl) for cl in range(NCLS)]

    tiles = []
    for t in range(T):
        rows = aso[RT * t:RT * t + RT]
        rcls = yso[RT * t:RT * t + RT]
        cls_order = list(dict.fromkeys(int(c) for c in rcls))
        assert len(cls_order) <= NLOC
        blockcols = np.concatenate([cols_by_class[cl] for cl in cls_order])
        assert len(blockcols) <= BLKW, len(blockcols)
        posmap = {int(cj): p for p, cj in enumerate(blockcols)}
        thr_t = np.array([posmap[int(a)] for a in rows], dtype=np.float32)
        lcls = {cl: lc for lc, cl in enumerate(cls_order)}
        tiles.append((rows, rcls, cls_order, blockcols, thr_t, lcls))

    iota93 = np.broadcast_to(np.arange(P, dtype=np.float32), (RT, P)).copy()
    colsel = np.zeros((Z, 256), dtype=np.float32)
    for ct in range(16):
        colsel[:, 16 * ct + ct] = 1.0

    in_maps = []
    for c in range(NCORE):
        zblks = [np.zeros((Z, BLKW), dtype=bf16) for _ in range(SLOTS)]
        yohb = np.zeros((NLOC, SLOTS * BLKW), dtype=bf16)
        zrlb = np.zeros((SLOTS * BLKW, Z), dtype=np.float32)
        pyb = np.zeros((NLOC + 1, SLOTS * RT), dtype=bf16)
        pyb[NLOC, :] = bf16(1.0)
        zatb = np.zeros((Z, SLOTS * RT), dtype=bf16)
        thr = np.zeros((RT, SLOTS), dtype=np.float32)
        for s in range(SLOTS):
            t = c + 8 * s
            if t >= T:
                continue
            rows, rcls, cls_order, blockcols, thr_t, lcls = tiles[t]
            nb = len(blockcols)
            nk = len(rows)
            zblks[s][:, :nb] = zTbf[:, blockcols]
            for p_, cj in enumerate(blockcols):
                yohb[lcls[int(yrel[cj])], s * BLKW + p_] = bf16(1.0)
            zrlb[s * BLKW:s * BLKW + nb, :] = z[blockcols, :]
            pyb[[lcls[int(cl)] for cl in rcls], s * RT + np.arange(nk)] = bf16(PEN)
            zatb[:, s * RT:s * RT + nk] = zTbf[:, rows]
            thr[:nk, s] = thr_t
        in_maps.append({
            "zblk0": zblks[0], "zblk1": zblks[1], "zblk2": zblks[2],
            "yohbl": yohb, "zrlb": zrlb, "pybl": pyb, "zatbl": zatb,
            "thrl": thr, "prx": np.asarray(proxies, dtype=np.float32),
            "iota93": iota93, "colsel": colsel,
        })
    return in_maps


def combine(results):
    total = 0.0
    for t in range(T):
        c, s = t % 8, t // 8
        nk = min(RT, A - RT * t)
        loss = results[c]["out"][:nk, 2 * s + 1].astype(np.float64)
        total += loss.sum()
    return np.float32(total / A)


def kernel(z, y_idx, proxies, y_map, _trace=False):
    if "nc" not in _CACHE:
        _CACHE["nc"] = build_program()
    nc = _CACHE["nc"]
    in_maps = prep_inputs(z, y_idx, proxies, y_map)
    res = run_bass_kernel_spmd(nc, in_maps, core_ids=list(range(NCORE)),
                               trace=_trace)
    out = combine(res.results)
    if _trace:
        return out, res
    return out


if __name__ == "__main__":
    import jax
    with jax.default_device(jax.devices("cpu")[0]):
        import reference
        inputs = {k: np.asarray(v) for k, v in reference.setup_inputs().items()}
        expected = np.asarray(jax.jit(reference.reference, backend="cpu")(**inputs))
    actual = kernel(**inputs)
    rel = abs(float(actual) - float(expected)) / max(abs(float(expected)), 1e-12)
    print(f"expected {expected}, actual {actual}, rel err {rel:.3e}")
